# revision 5
# baseline (speedup 1.0000x reference)
"""CLIP-style contrastive (NT-Xent) loss on 8 Trainium2 NeuronCores.

Strategy (data-parallel, per sharding hint):
  - Shard the batch (4096) across 8 cores: 512 rows of x_image/x_text each.
  - Each core projects its shard through both towers in TRANSPOSED
    activation layout ([feat_partitions, batch_free]) so every Linear uses
    the stored weight directly as the stationary lhsT (out = lhsT.T @ rhs)
    and no activation transposes are ever needed.  Inputs are fed
    pre-transposed (xT shards, prepared host-side during sharding) and
    pre-cast to bf16 for the TensorEngine.
  - L2-normalize the 128-dim projections on-device (fp32), AllGather the
    bf16 normalized projections across all 8 cores (one AG per modality so
    the image AG overlaps the text tower's compute).
  - Each core computes its 1024 rows of the global 8192x8192 similarity
    matrix in [128, 2048] PSUM chunks (bf16 matmuls, fp32 accumulate),
    applies exp(sim/t) on ScalarE with fused per-row accumulation
    (accum_out), giving row sums T_r that include the self-similarity
    diagonal.
  - Device returns, per row: T_r, diag_r (=|z_r|^2 of the normalized z,
    computed exactly like the matmul does), and pos_r (= z1_b . z2_b).
    Host finishes in fp64:
        T'_r   = T_r - exp(diag_r/t) + exp(pos_r/t)
        loss_r = log(T'_r) - pos_r/t
    which matches the reference exactly (the reference's logits contain
    the positive twice: once as logits[:,0] and once among "negatives").
"""

import numpy as np
import ml_dtypes

import concourse.bacc as bacc
import concourse.bass as bass
import concourse.mybir as mybir
import concourse.tile as tile
from concourse.bass_utils import run_bass_kernel_spmd

NCORES = 8
B, DIN, DE, DH, DP = 4096, 1024, 512, 256, 128
S = B // NCORES            # 512: per-core batch shard
ROWS = 2 * S               # 1024 sim rows owned per core (z1 + z2 shard)
N = 2 * B                  # 8192 global rows
TEMP = 0.07
INV_T = 1.0 / TEMP

F32 = mybir.dt.float32
BF16 = mybir.dt.bfloat16
SIM_DT = BF16              # dtype of the similarity matmul operands
PROJ_DT = BF16             # dtype of projection matmul operands
NP_PROJ = ml_dtypes.bfloat16 if PROJ_DT == BF16 else np.float32

# device output layout: [128, 20] = T(8) | pos(4) | diag_img(4) | diag_txt(4)
OUT_COLS = 20

_CACHE: dict = {}


def _build():
    nc = bacc.Bacc("TRN2", target_bir_lowering=False, debug=False,
                   num_devices=NCORES)

    t_in = {}
    for m in ("img", "txt"):
        t_in[f"xT_{m}"] = nc.dram_tensor(f"xT_{m}", [DIN, S], PROJ_DT,
                                         kind="ExternalInput")
        t_in[f"We_{m}"] = nc.dram_tensor(f"We_{m}", [DIN, DE], PROJ_DT,
                                         kind="ExternalInput")
        t_in[f"Wp1_{m}"] = nc.dram_tensor(f"Wp1_{m}", [DE, DH], PROJ_DT,
                                          kind="ExternalInput")
        t_in[f"Wp2_{m}"] = nc.dram_tensor(f"Wp2_{m}", [DH, DP], PROJ_DT,
                                          kind="ExternalInput")
        t_in[f"beT_{m}"] = nc.dram_tensor(f"beT_{m}", [128, DE // 128], F32,
                                          kind="ExternalInput")
        t_in[f"bp1T_{m}"] = nc.dram_tensor(f"bp1T_{m}", [128, DH // 128], F32,
                                           kind="ExternalInput")
        t_in[f"bp2T_{m}"] = nc.dram_tensor(f"bp2T_{m}", [128, DP // 128], F32,
                                           kind="ExternalInput")
    out_t = nc.dram_tensor("parts", [128, OUT_COLS], F32,
                           kind="ExternalOutput")

    with tile.TileContext(nc) as tc:
        _emit(nc, tc, t_in, out_t)
    nc.compile()
    return nc


def _load_weights(nc, wpool, t_in, m):
    """DMA one tower's operands; xT/We chunks interleaved so the L1 k-loop
    can start as soon as the first pair lands."""
    xt = wpool.tile([128, (DIN // 128) * S], PROJ_DT, name=f"xt_{m}")
    we = wpool.tile([128, (DIN // 128) * DE], PROJ_DT, name=f"we_{m}")
    for k in range(DIN // 128):
        nc.sync.dma_start(out=we[:, k * DE:(k + 1) * DE],
                          in_=t_in[f"We_{m}"][128 * k:128 * (k + 1), :])
        nc.sync.dma_start(out=xt[:, k * S:(k + 1) * S],
                          in_=t_in[f"xT_{m}"][128 * k:128 * (k + 1), :])
    wp1 = wpool.tile([128, (DE // 128) * DH], PROJ_DT, name=f"wp1_{m}")
    for k in range(DE // 128):
        nc.sync.dma_start(out=wp1[:, k * DH:(k + 1) * DH],
                          in_=t_in[f"Wp1_{m}"][128 * k:128 * (k + 1), :])
    wp2 = wpool.tile([128, (DH // 128) * DP], PROJ_DT, name=f"wp2_{m}")
    for k in range(DH // 128):
        nc.sync.dma_start(out=wp2[:, k * DP:(k + 1) * DP],
                          in_=t_in[f"Wp2_{m}"][128 * k:128 * (k + 1), :])
    beT = wpool.tile([128, DE // 128], F32, name=f"beT_{m}")
    nc.sync.dma_start(out=beT[:], in_=t_in[f"beT_{m}"][:, :])
    bp1T = wpool.tile([128, DH // 128], F32, name=f"bp1T_{m}")
    nc.sync.dma_start(out=bp1T[:], in_=t_in[f"bp1T_{m}"][:, :])
    bp2T = wpool.tile([128, DP // 128], F32, name=f"bp2T_{m}")
    nc.sync.dma_start(out=bp2T[:], in_=t_in[f"bp2T_{m}"][:, :])
    return dict(xt=xt, we=we, wp1=wp1, wp2=wp2, beT=beT, bp1T=bp1T, bp2T=bp2T)


def _project_normalize(nc, tc, pps, psb, apool, w, m, ones_col, ones_row):
    """Emit one tower: projections (bf16 matmuls) + fp32 normalize.

    Returns (zn fp32 [128,512], znb SIM_DT [128,512])."""
    Exp = mybir.ActivationFunctionType.Exp
    Ln = mybir.ActivationFunctionType.Ln
    add = mybir.AluOpType.add
    mx = mybir.AluOpType.max

    h = psb.tile([128, (DE // 128) * S], PROJ_DT, tag="h")
    for mm in range(DE // 128):
        ph = pps.tile([128, S], F32, tag="simps")
        for k in range(DIN // 128):
            nc.tensor.matmul(
                ph[:],
                w["we"][:, k * DE + 128 * mm: k * DE + 128 * (mm + 1)],
                w["xt"][:, k * S:(k + 1) * S],
                start=(k == 0), stop=(k == DIN // 128 - 1))
        nc.vector.tensor_scalar(
            out=h[:, mm * S:(mm + 1) * S], in0=ph[:],
            scalar1=w["beT"][:, mm:mm + 1], scalar2=None, op0=add)
    g = psb.tile([128, (DH // 128) * S], PROJ_DT, tag="g")
    for mm in range(DH // 128):
        pg = pps.tile([128, S], F32, tag="simps")
        for k in range(DE // 128):
            nc.tensor.matmul(
                pg[:],
                w["wp1"][:, k * DH + 128 * mm: k * DH + 128 * (mm + 1)],
                h[:, k * S:(k + 1) * S],
                start=(k == 0), stop=(k == DE // 128 - 1))
        nc.vector.tensor_scalar(
            out=g[:, mm * S:(mm + 1) * S], in0=pg[:],
            scalar1=w["bp1T"][:, mm:mm + 1], scalar2=0.0, op0=add, op1=mx)
    pz = pps.tile([128, S], F32, tag="simps")
    for k in range(DH // 128):
        nc.tensor.matmul(pz[:], w["wp2"][:, k * DP: k * DP + 128],
                         g[:, k * S:(k + 1) * S],
                         start=(k == 0), stop=(k == DH // 128 - 1))
    z = psb.tile([128, S], F32, tag=f"z_{m}")
    nc.vector.tensor_scalar(out=z[:], in0=pz[:], scalar1=w["bp2T"][:, 0:1],
                            scalar2=None, op0=add)

    # normalize columns (rows of z): inv = exp(-0.5 * ln(sum z^2))
    sq = psb.tile([128, S], F32, tag="sq")
    nc.vector.tensor_mul(sq[:], z[:], z[:])
    pssq = pps.tile([1, S], F32, tag="simps")
    nc.tensor.matmul(pssq[:], ones_col[:], sq[:], start=True, stop=True)
    lnr = psb.tile([1, S], F32, tag="lnr")
    nc.scalar.activation(lnr[:], pssq[:], Ln)
    inv = psb.tile([1, S], F32, tag="inv")
    nc.scalar.activation(inv[:], lnr[:], Exp, scale=-0.5)
    pinvb = pps.tile([128, S], F32, tag="simps")
    nc.tensor.matmul(pinvb[:], ones_row[:], inv[:], start=True, stop=True)
    zn = apool.tile([128, S], F32, name=f"zn_{m}")
    nc.vector.tensor_mul(zn[:], z[:], pinvb[:])
    znb = apool.tile([128, S], SIM_DT, name=f"znb_{m}")
    nc.vector.tensor_copy(znb[:], zn[:])
    return zn, znb


def _emit(nc, tc, t_in, out_t):
    Exp = mybir.ActivationFunctionType.Exp
    add = mybir.AluOpType.add

    NCHUNK = 2048                  # columns per PSUM super-chunk (4 banks)
    NTT = N // NCHUNK              # 4
    NRC = ROWS // 128              # 8 row chunks

    with tc.tile_pool(name="const", bufs=1) as cpool, \
         tc.tile_pool(name="wpool", bufs=1) as wpool, \
         tc.tile_pool(name="actpool", bufs=1) as apool, \
         tc.tile_pool(name="projsb", bufs=2) as psb, \
         tc.tile_pool(name="psum", bufs=2, space="PSUM") as pps, \
         tc.tile_pool(name="escp", bufs=2) as escp, \
         tc.tile_pool(name="dram", bufs=1, space="DRAM") as dram:

        ones_col = cpool.tile([128, 1], F32)
        nc.any.memset(ones_col[:], 1.0)
        ones_row = cpool.tile([1, 128], F32)
        nc.any.memset(ones_row[:], 1.0)
        warm_rhs = cpool.tile([128, S], F32)
        nc.vector.memset(warm_rhs[:], 0.0)

        # HAM warm-up: ~8us of dummy matmuls so the projection phase runs
        # at 2.4 GHz instead of the 1.2 GHz cold clock.
        warm_ps = pps.tile([128, S], F32, tag="simps")
        for _ in range(20):
            nc.tensor.matmul(warm_ps[:], ones_row[:], warm_rhs[0:1, :],
                             start=True, stop=True)

        # prefetch BOTH towers' operands up front (sync queue, no stalls)
        w_all = {m: _load_weights(nc, wpool, t_in, m) for m in ("img", "txt")}

        zn, znb, cc_out = {}, {}, {}
        zf = {"img": apool.tile([128, B], SIM_DT, name="zf_img"),
              "txt": apool.tile([128, B], SIM_DT, name="zf_txt")}
        for m in ("img", "txt"):
            zn[m], znb[m] = _project_normalize(
                nc, tc, pps, psb, apool, w_all[m], m, ones_col, ones_row)
            # AllGather this modality right away (img AG overlaps txt tower)
            cc_in = dram.tile([128, S], SIM_DT, name=f"cc_in_{m}")
            nc.sync.dma_start(out=cc_in[:, :], in_=znb[m][:])
            cc_o = dram.tile([128 * NCORES, S], SIM_DT, name=f"cc_out_{m}",
                             addr_space="Shared")
            nc.gpsimd.collective_compute(
                "AllGather", mybir.AluOpType.bypass,
                replica_groups=[list(range(NCORES))],
                ins=[cc_in[:]], outs=[cc_o[:]])
            cc_out[m] = cc_o

        # pos / self-diag rows ([1, 512] each) -> [128, 4] via DRAM scatter
        rows_d = dram.tile([3, S], F32)
        for r, (a, b) in enumerate((("img", "txt"), ("img", "img"),
                                    ("txt", "txt"))):
            prod = psb.tile([128, S], F32, tag="sq")
            nc.vector.tensor_mul(prod[:], zn[a][:], zn[b][:])
            pr = pps.tile([1, S], F32, tag="simps")
            nc.tensor.matmul(pr[:], ones_col[:], prod[:], start=True,
                             stop=True)
            row_sb = psb.tile([1, S], F32, tag="rowsb")
            nc.vector.tensor_copy(row_sb[:], pr[:])
            nc.sync.dma_start(out=rows_d[r:r + 1, :], in_=row_sb[:])

        pdT = apool.tile([128, 12], F32)   # pos | diag_img | diag_txt
        for r in range(3):
            nc.sync.dma_start(
                out=pdT[:, 4 * r:4 * (r + 1)],
                in_=rows_d[r:r + 1, :].rearrange("o (c p) -> (o p) c", p=128))

        # gathered projections -> SBUF (emitted after all small DMAs so the
        # sync-queue FIFO has nothing queued behind these AG-gated loads)
        for m in ("img", "txt"):
            for j in range(NCORES):
                nc.sync.dma_start(
                    out=zf[m][:, S * j: S * (j + 1)],
                    in_=cc_out[m][128 * j:128 * (j + 1), :])

        # ---- main loop: sim rows + exp + fused row sums ----
        # image columns (ready after AG1) run before text columns.
        stats = apool.tile([128, NRC * NTT], F32)
        for tt in range(NTT):
            src = zf["img"] if tt < NTT // 2 else zf["txt"]
            coff = (tt % (NTT // 2)) * NCHUNK
            for rc in range(NRC):
                if rc < 4:
                    lhs = znb["img"][:, 128 * rc:128 * (rc + 1)]
                else:
                    lhs = znb["txt"][:, 128 * (rc - 4):128 * (rc - 3)]
                ps = pps.tile([128, NCHUNK], F32, tag="simps")
                for q in range(NCHUNK // 512):
                    nc.tensor.matmul(
                        ps[:, 512 * q:512 * (q + 1)], lhs,
                        src[:, coff + 512 * q: coff + 512 * (q + 1)],
                        start=True, stop=True)
                esc = escp.tile([128, NCHUNK], F32, tag="esc")
                nc.scalar.activation(
                    esc[:], ps[:], Exp, scale=INV_T,
                    accum_out=stats[:, NTT * rc + tt: NTT * rc + tt + 1])

        # ---- gather outputs: T (8) | pos(4) | diag1(4) | diag2(4) ----
        outv = apool.tile([128, OUT_COLS], F32)
        nc.vector.tensor_reduce(
            out=outv[:, 0:NRC],
            in_=stats[:].rearrange("p (r t) -> p r t", t=NTT),
            axis=mybir.AxisListType.X, op=add)
        nc.vector.tensor_copy(outv[:, NRC:NRC + 12], pdT[:])
        nc.sync.dma_start(out=out_t[:, :], in_=outv[:])


def _prep_in_maps(inputs):
    host = {}
    for m in ("img", "txt"):
        host[f"We_{m}"] = np.ascontiguousarray(inputs[f"We_{m}"]).astype(NP_PROJ)
        host[f"Wp1_{m}"] = np.ascontiguousarray(inputs[f"Wp1_{m}"]).astype(NP_PROJ)
        host[f"Wp2_{m}"] = np.ascontiguousarray(inputs[f"Wp2_{m}"]).astype(NP_PROJ)
        host[f"beT_{m}"] = np.ascontiguousarray(
            np.asarray(inputs[f"be_{m}"], np.float32).reshape(DE // 128, 128).T)
        host[f"bp1T_{m}"] = np.ascontiguousarray(
            np.asarray(inputs[f"bp1_{m}"], np.float32).reshape(DH // 128, 128).T)
        host[f"bp2T_{m}"] = np.ascontiguousarray(
            np.asarray(inputs[f"bp2_{m}"], np.float32).reshape(DP // 128, 128).T)
    x = {"img": np.asarray(inputs["x_image"], np.float32),
         "txt": np.asarray(inputs["x_text"], np.float32)}
    in_maps = []
    for c in range(NCORES):
        mp = dict(host)
        for m in ("img", "txt"):
            mp[f"xT_{m}"] = np.ascontiguousarray(
                x[m][c * S:(c + 1) * S].T).astype(NP_PROJ)
        in_maps.append(mp)
    return in_maps


def _finish_host(results):
    """Host-side fp64 finish: combine per-core T/pos/diag into the loss."""
    total = 0.0
    t = TEMP
    for c in range(NCORES):
        p = np.asarray(results[c]["parts"], np.float64)
        T = p[:, 0:8]           # [128, rc]
        pos = p[:, 8:12]        # [128, k]  (k = batch chunk within shard)
        d1 = p[:, 12:16]
        d2 = p[:, 16:20]
        for rc in range(8):
            k = rc % 4
            dg = d1[:, k] if rc < 4 else d2[:, k]
            Tp = T[:, rc] - np.exp(dg / t) + np.exp(pos[:, k] / t)
            total += float(np.sum(np.log(Tp) - pos[:, k] / t))
    return np.float32(total / N)


def kernel(**inputs) -> np.ndarray:
    nc = _CACHE.get("nc")
    if nc is None:
        nc = _build()
        _CACHE["nc"] = nc
    res = run_bass_kernel_spmd(nc, _prep_in_maps(inputs),
                               core_ids=list(range(NCORES)))
    return _finish_host(res.results)


# revision 6
# speedup vs baseline: 1.1150x; 1.1150x over previous
"""CLIP-style contrastive (NT-Xent) loss on 8 Trainium2 NeuronCores.

Strategy (data-parallel, per sharding hint):
  - Shard the batch (4096) across 8 cores: 512 rows of x_image/x_text each.
  - Each core projects its shard through both towers in TRANSPOSED
    activation layout ([feat_partitions, batch_free]) so every Linear uses
    the stored weight directly as the stationary lhsT (out = lhsT.T @ rhs)
    and no activation transposes are ever needed.  Inputs are fed
    pre-transposed (xT shards, prepared host-side during sharding) and
    pre-cast to bf16 for the TensorEngine.
  - L2-normalize the 128-dim projections on-device (fp32), AllGather the
    bf16 normalized projections across all 8 cores (one AG per modality so
    the image AG overlaps the text tower's compute).
  - Each core computes its 1024 rows of the global 8192x8192 similarity
    matrix in [128, 2048] PSUM chunks (bf16 matmuls, fp32 accumulate),
    applies exp(sim/t) on ScalarE with fused per-row accumulation
    (accum_out), giving row sums T_r that include the self-similarity
    diagonal.
  - Device returns, per row: T_r, diag_r (=|z_r|^2 of the normalized z,
    computed exactly like the matmul does), and pos_r (= z1_b . z2_b).
    Host finishes in fp64:
        T'_r   = T_r - exp(diag_r/t) + exp(pos_r/t)
        loss_r = log(T'_r) - pos_r/t
    which matches the reference exactly (the reference's logits contain
    the positive twice: once as logits[:,0] and once among "negatives").
"""

import numpy as np
import ml_dtypes

import concourse.bacc as bacc
import concourse.bass as bass
import concourse.mybir as mybir
import concourse.tile as tile
from concourse.bass_utils import run_bass_kernel_spmd

NCORES = 8
B, DIN, DE, DH, DP = 4096, 1024, 512, 256, 128
S = B // NCORES            # 512: per-core batch shard
ROWS = 2 * S               # 1024 sim rows owned per core (z1 + z2 shard)
N = 2 * B                  # 8192 global rows
TEMP = 0.07
INV_T = 1.0 / TEMP

F32 = mybir.dt.float32
BF16 = mybir.dt.bfloat16
SIM_DT = BF16              # dtype of the similarity matmul operands
PROJ_DT = BF16             # dtype of projection matmul operands
NP_PROJ = ml_dtypes.bfloat16 if PROJ_DT == BF16 else np.float32

# device output layout: [128, 20] = T(8) | pos(4) | diag_img(4) | diag_txt(4)
OUT_COLS = 20

_CACHE: dict = {}


def _build():
    nc = bacc.Bacc("TRN2", target_bir_lowering=False, debug=False,
                   num_devices=NCORES)

    t_in = {}
    for m in ("img", "txt"):
        t_in[f"xT_{m}"] = nc.dram_tensor(f"xT_{m}", [DIN, S], PROJ_DT,
                                         kind="ExternalInput")
        t_in[f"We_{m}"] = nc.dram_tensor(f"We_{m}", [DIN, DE], PROJ_DT,
                                         kind="ExternalInput")
        t_in[f"Wp1_{m}"] = nc.dram_tensor(f"Wp1_{m}", [DE, DH], PROJ_DT,
                                          kind="ExternalInput")
        t_in[f"Wp2_{m}"] = nc.dram_tensor(f"Wp2_{m}", [DH, DP], PROJ_DT,
                                          kind="ExternalInput")
        t_in[f"beT_{m}"] = nc.dram_tensor(f"beT_{m}", [128, DE // 128], F32,
                                          kind="ExternalInput")
        t_in[f"bp1T_{m}"] = nc.dram_tensor(f"bp1T_{m}", [128, DH // 128], F32,
                                           kind="ExternalInput")
        t_in[f"bp2T_{m}"] = nc.dram_tensor(f"bp2T_{m}", [128, DP // 128], F32,
                                           kind="ExternalInput")
    out_t = nc.dram_tensor("parts", [128, OUT_COLS], F32,
                           kind="ExternalOutput")

    with tile.TileContext(nc) as tc:
        _emit(nc, tc, t_in, out_t)
    nc.compile()
    return nc


def _load_weights(nc, wpool, t_in, m):
    """DMA one tower's operands; xT/We chunks interleaved so the L1 k-loop
    can start as soon as the first pair lands."""
    xt = wpool.tile([128, (DIN // 128) * S], PROJ_DT, name=f"xt_{m}")
    we = wpool.tile([128, (DIN // 128) * DE], PROJ_DT, name=f"we_{m}")
    for k in range(DIN // 128):
        nc.sync.dma_start(out=we[:, k * DE:(k + 1) * DE],
                          in_=t_in[f"We_{m}"][128 * k:128 * (k + 1), :])
        nc.sync.dma_start(out=xt[:, k * S:(k + 1) * S],
                          in_=t_in[f"xT_{m}"][128 * k:128 * (k + 1), :])
    wp1 = wpool.tile([128, (DE // 128) * DH], PROJ_DT, name=f"wp1_{m}")
    for k in range(DE // 128):
        nc.sync.dma_start(out=wp1[:, k * DH:(k + 1) * DH],
                          in_=t_in[f"Wp1_{m}"][128 * k:128 * (k + 1), :])
    wp2 = wpool.tile([128, (DH // 128) * DP], PROJ_DT, name=f"wp2_{m}")
    for k in range(DH // 128):
        nc.sync.dma_start(out=wp2[:, k * DP:(k + 1) * DP],
                          in_=t_in[f"Wp2_{m}"][128 * k:128 * (k + 1), :])
    beT = wpool.tile([128, DE // 128], F32, name=f"beT_{m}")
    nc.sync.dma_start(out=beT[:], in_=t_in[f"beT_{m}"][:, :])
    bp1T = wpool.tile([128, DH // 128], F32, name=f"bp1T_{m}")
    nc.sync.dma_start(out=bp1T[:], in_=t_in[f"bp1T_{m}"][:, :])
    bp2T = wpool.tile([128, DP // 128], F32, name=f"bp2T_{m}")
    nc.sync.dma_start(out=bp2T[:], in_=t_in[f"bp2T_{m}"][:, :])
    return dict(xt=xt, we=we, wp1=wp1, wp2=wp2, beT=beT, bp1T=bp1T, bp2T=bp2T)


def _project_normalize(nc, tc, pps, psb, apool, w, m, ones_col, ones_row):
    """Emit one tower: projections (bf16 matmuls) + fp32 normalize.

    Returns (zn fp32 [128,512], znb SIM_DT [128,512])."""
    Exp = mybir.ActivationFunctionType.Exp
    Ln = mybir.ActivationFunctionType.Ln
    add = mybir.AluOpType.add
    mx = mybir.AluOpType.max

    h = psb.tile([128, (DE // 128) * S], PROJ_DT, tag="h")
    for mm in range(DE // 128):
        ph = pps.tile([128, S], F32, tag="simps")
        for k in range(DIN // 128):
            nc.tensor.matmul(
                ph[:],
                w["we"][:, k * DE + 128 * mm: k * DE + 128 * (mm + 1)],
                w["xt"][:, k * S:(k + 1) * S],
                start=(k == 0), stop=(k == DIN // 128 - 1))
        nc.vector.tensor_scalar(
            out=h[:, mm * S:(mm + 1) * S], in0=ph[:],
            scalar1=w["beT"][:, mm:mm + 1], scalar2=None, op0=add)
    g = psb.tile([128, (DH // 128) * S], PROJ_DT, tag="g")
    for mm in range(DH // 128):
        pg = pps.tile([128, S], F32, tag="simps")
        for k in range(DE // 128):
            nc.tensor.matmul(
                pg[:],
                w["wp1"][:, k * DH + 128 * mm: k * DH + 128 * (mm + 1)],
                h[:, k * S:(k + 1) * S],
                start=(k == 0), stop=(k == DE // 128 - 1))
        nc.vector.tensor_scalar(
            out=g[:, mm * S:(mm + 1) * S], in0=pg[:],
            scalar1=w["bp1T"][:, mm:mm + 1], scalar2=0.0, op0=add, op1=mx)
    pz = pps.tile([128, S], F32, tag="simps")
    for k in range(DH // 128):
        nc.tensor.matmul(pz[:], w["wp2"][:, k * DP: k * DP + 128],
                         g[:, k * S:(k + 1) * S],
                         start=(k == 0), stop=(k == DH // 128 - 1))
    z = psb.tile([128, S], F32, tag=f"z_{m}")
    nc.vector.tensor_scalar(out=z[:], in0=pz[:], scalar1=w["bp2T"][:, 0:1],
                            scalar2=None, op0=add)

    # normalize columns (rows of z): inv = exp(-0.5 * ln(sum z^2))
    sq = psb.tile([128, S], F32, tag="sq")
    nc.vector.tensor_mul(sq[:], z[:], z[:])
    pssq = pps.tile([1, S], F32, tag="simps")
    nc.tensor.matmul(pssq[:], ones_col[:], sq[:], start=True, stop=True)
    lnr = psb.tile([1, S], F32, tag="lnr")
    nc.scalar.activation(lnr[:], pssq[:], Ln)
    inv = psb.tile([1, S], F32, tag="inv")
    nc.scalar.activation(inv[:], lnr[:], Exp, scale=-0.5)
    pinvb = pps.tile([128, S], F32, tag="simps")
    nc.tensor.matmul(pinvb[:], ones_row[:], inv[:], start=True, stop=True)
    zn = apool.tile([128, S], F32, name=f"zn_{m}")
    nc.vector.tensor_mul(zn[:], z[:], pinvb[:])
    znb = apool.tile([128, S], SIM_DT, name=f"znb_{m}")
    nc.vector.tensor_copy(znb[:], zn[:])
    return zn, znb


def _emit(nc, tc, t_in, out_t):
    Exp = mybir.ActivationFunctionType.Exp
    add = mybir.AluOpType.add

    NCHUNK = 2048                  # columns per PSUM super-chunk (4 banks)
    NTT = N // NCHUNK              # 4
    NRC = ROWS // 128              # 8 row chunks

    with tc.tile_pool(name="const", bufs=1) as cpool, \
         tc.tile_pool(name="wpool", bufs=1) as wpool, \
         tc.tile_pool(name="actpool", bufs=1) as apool, \
         tc.tile_pool(name="projsb", bufs=2) as psb, \
         tc.tile_pool(name="psum", bufs=2, space="PSUM") as pps, \
         tc.tile_pool(name="escp", bufs=2) as escp, \
         tc.tile_pool(name="dram", bufs=1, space="DRAM") as dram:

        ones_col = cpool.tile([128, 1], F32)
        nc.any.memset(ones_col[:], 1.0)
        ones_row = cpool.tile([1, 128], F32)
        nc.any.memset(ones_row[:], 1.0)
        # prefetch BOTH towers' operands up front (sync queue, no stalls)
        w_all = {m: _load_weights(nc, wpool, t_in, m) for m in ("img", "txt")}

        zn, znb, cc_out = {}, {}, {}
        zf = {"img": apool.tile([128, B], SIM_DT, name="zf_img"),
              "txt": apool.tile([128, B], SIM_DT, name="zf_txt")}
        for m in ("img", "txt"):
            zn[m], znb[m] = _project_normalize(
                nc, tc, pps, psb, apool, w_all[m], m, ones_col, ones_row)
            # AllGather this modality right away (img AG overlaps txt tower).
            # Bounce DMA rides the ScalarE HWDGE queue so the sync queue's
            # AG-gated zf loads can't head-of-line block the second trigger.
            cc_in = dram.tile([128, S], SIM_DT, name=f"cc_in_{m}")
            nc.scalar.dma_start(out=cc_in[:, :], in_=znb[m][:])
            cc_o = dram.tile([128 * NCORES, S], SIM_DT, name=f"cc_out_{m}",
                             addr_space="Shared")
            nc.gpsimd.collective_compute(
                "AllGather", mybir.AluOpType.bypass,
                replica_groups=[list(range(NCORES))],
                ins=[cc_in[:]], outs=[cc_o[:]])
            cc_out[m] = cc_o
            for j in range(NCORES):
                nc.sync.dma_start(
                    out=zf[m][:, S * j: S * (j + 1)],
                    in_=cc_o[128 * j:128 * (j + 1), :])

        # pos / self-diag rows ([1, 512] each) -> [128, 4] via DRAM scatter
        rows_d = dram.tile([3, S], F32)
        for r, (a, b) in enumerate((("img", "txt"), ("img", "img"),
                                    ("txt", "txt"))):
            prod = psb.tile([128, S], F32, tag="sq")
            nc.vector.tensor_mul(prod[:], zn[a][:], zn[b][:])
            pr = pps.tile([1, S], F32, tag="simps")
            nc.tensor.matmul(pr[:], ones_col[:], prod[:], start=True,
                             stop=True)
            row_sb = psb.tile([1, S], F32, tag="rowsb")
            nc.vector.tensor_copy(row_sb[:], pr[:])
            nc.sync.dma_start(out=rows_d[r:r + 1, :], in_=row_sb[:])

        pdT = apool.tile([128, 12], F32)   # pos | diag_img | diag_txt
        for r in range(3):
            nc.sync.dma_start(
                out=pdT[:, 4 * r:4 * (r + 1)],
                in_=rows_d[r:r + 1, :].rearrange("o (c p) -> (o p) c", p=128))

        # ---- main loop: sim rows + exp + fused row sums ----
        # image columns (ready after AG1) run before text columns.
        stats = apool.tile([128, NRC * NTT], F32)
        for tt in range(NTT):
            src = zf["img"] if tt < NTT // 2 else zf["txt"]
            coff = (tt % (NTT // 2)) * NCHUNK
            for rc in range(NRC):
                if rc < 4:
                    lhs = znb["img"][:, 128 * rc:128 * (rc + 1)]
                else:
                    lhs = znb["txt"][:, 128 * (rc - 4):128 * (rc - 3)]
                ps = pps.tile([128, NCHUNK], F32, tag="simps")
                for q in range(NCHUNK // 512):
                    nc.tensor.matmul(
                        ps[:, 512 * q:512 * (q + 1)], lhs,
                        src[:, coff + 512 * q: coff + 512 * (q + 1)],
                        start=True, stop=True)
                esc = escp.tile([128, NCHUNK], F32, tag="esc")
                nc.scalar.activation(
                    esc[:], ps[:], Exp, scale=INV_T,
                    accum_out=stats[:, NTT * rc + tt: NTT * rc + tt + 1])

        # ---- gather outputs: T (8) | pos(4) | diag1(4) | diag2(4) ----
        outv = apool.tile([128, OUT_COLS], F32)
        nc.vector.tensor_reduce(
            out=outv[:, 0:NRC],
            in_=stats[:].rearrange("p (r t) -> p r t", t=NTT),
            axis=mybir.AxisListType.X, op=add)
        nc.vector.tensor_copy(outv[:, NRC:NRC + 12], pdT[:])
        nc.sync.dma_start(out=out_t[:, :], in_=outv[:])


def _prep_in_maps(inputs):
    host = {}
    for m in ("img", "txt"):
        host[f"We_{m}"] = np.ascontiguousarray(inputs[f"We_{m}"]).astype(NP_PROJ)
        host[f"Wp1_{m}"] = np.ascontiguousarray(inputs[f"Wp1_{m}"]).astype(NP_PROJ)
        host[f"Wp2_{m}"] = np.ascontiguousarray(inputs[f"Wp2_{m}"]).astype(NP_PROJ)
        host[f"beT_{m}"] = np.ascontiguousarray(
            np.asarray(inputs[f"be_{m}"], np.float32).reshape(DE // 128, 128).T)
        host[f"bp1T_{m}"] = np.ascontiguousarray(
            np.asarray(inputs[f"bp1_{m}"], np.float32).reshape(DH // 128, 128).T)
        host[f"bp2T_{m}"] = np.ascontiguousarray(
            np.asarray(inputs[f"bp2_{m}"], np.float32).reshape(DP // 128, 128).T)
    x = {"img": np.asarray(inputs["x_image"], np.float32),
         "txt": np.asarray(inputs["x_text"], np.float32)}
    in_maps = []
    for c in range(NCORES):
        mp = dict(host)
        for m in ("img", "txt"):
            mp[f"xT_{m}"] = np.ascontiguousarray(
                x[m][c * S:(c + 1) * S].T).astype(NP_PROJ)
        in_maps.append(mp)
    return in_maps


def _finish_host(results):
    """Host-side fp64 finish: combine per-core T/pos/diag into the loss."""
    total = 0.0
    t = TEMP
    for c in range(NCORES):
        p = np.asarray(results[c]["parts"], np.float64)
        T = p[:, 0:8]           # [128, rc]
        pos = p[:, 8:12]        # [128, k]  (k = batch chunk within shard)
        d1 = p[:, 12:16]
        d2 = p[:, 16:20]
        for rc in range(8):
            k = rc % 4
            dg = d1[:, k] if rc < 4 else d2[:, k]
            Tp = T[:, rc] - np.exp(dg / t) + np.exp(pos[:, k] / t)
            total += float(np.sum(np.log(Tp) - pos[:, k] / t))
    return np.float32(total / N)


def kernel(**inputs) -> np.ndarray:
    nc = _CACHE.get("nc")
    if nc is None:
        nc = _build()
        _CACHE["nc"] = nc
    res = run_bass_kernel_spmd(nc, _prep_in_maps(inputs),
                               core_ids=list(range(NCORES)))
    return _finish_host(res.results)
